# revision 2
# baseline (speedup 1.0000x reference)
"""Multi-head self-attention (8 equal segments of 1024 tokens) on 8 TRN2 cores.

Sharding: one segment per core; projection weights replicated.

v2 design (fp8 + engine-balanced softmax):
  x arrives bf16 (host-converted). PE transposes x -> xT (bf16,
  feature-major); DVE converts xT -> xT8 (fp8e4).
  Wq/Wk/Wv are fp8e4, host-scaled by 32 (keeps entries out of fp8
  subnormals); Wc is bf16.
  q/k: fp8 DoubleRow matmuls (K=256/instr) -> qT8/kT8 fp8e4 feature-major.
  v:   fp8 DoubleRow token-major -> vo5 fp8e4 with a 32.0 column per head
       (row 64 accumulates 32*sum(exp) during PV; the 32s cancel in the
       normalize divide).
  scores: plain fp8 matmuls, K=64, head pairs on disjoint PE row groups.
  softmax: psc holds 8192*z (z = q.k/8 in true scale). Split per-unit
       between ACT (true exp -> fp8e4, scale 1/8192, bias -2) and DVE
       (Schraudolph: int8 <- psc*A + B rounds to an e5m2 bit pattern of
       ~exp(z-2); bitcast). The -2 shift keeps exp below fp8 max and
       cancels in the normalize.
  PV: fp8 DoubleRow, feature-major po[65, 512] (row 64 = 32*sum p).
  po -> ot (bf16) -> PE transpose -> ptr [128q, 8, 65] -> reciprocal of
  col 64 -> broadcast-multiply -> attn_sb (bf16 token-major).
  c: bf16 matmuls (x@Wc); consume adds attn_sb and DMAs out.
  All PSUM->SBUF drains are assigned greedily to ACT/DVE to balance the
  two engines (softmax exp dominates; the rest fills the gaps).
"""

import numpy as np
import ml_dtypes

import concourse.mybir as mybir
import concourse.tile as tile
from concourse import bacc
from concourse.bass_utils import run_bass_kernel_spmd
from concourse.masks import make_identity

P = 128
S = 1024
D = 1024
H = 16
W = 64
NCORES = 8
KO = D // P      # 8 contraction chunks
KP = KO // 2     # 4 DoubleRow chunk-pairs
TO = S // P      # 8 token chunks
NJ = S // 512    # 2 q free-dim chunks
WSCALE = 32.0

f32 = mybir.dt.float32
f32r = mybir.dt.float32r
bf16 = mybir.dt.bfloat16
fp8e4 = mybir.dt.float8e4
fp8e5 = mybir.dt.float8e5
u8 = mybir.dt.uint8

_ACT_EXP = mybir.ActivationFunctionType.Exp
_ALU_ADD = mybir.AluOpType.add
_ALU_MULT = mybir.AluOpType.mult
_DR = mybir.MatmulPerfMode.DoubleRow

# softmax scaling: psc = 8192 * z where z = true scaled score (q.k/sqrt(64))
# Engine assignment is per (h, j) so each softmax row uses ONE engine and
# per-engine exp shifts cancel row-wise in the normalize:
#   ACT: exp(z-4) in e4m3 (range cap 240 -> safe to z=9.4)
#   DVE: Schraudolph uint8 code of exp(z-2) in e5m2 (negatives saturate
#        to 0 -> prob 0; code stays < 124 for z < 12.9)
_PSC_SCALE = 1.0 / (WSCALE * WSCALE * 8.0)     # 1/8192
_EXP_SHIFT_ACT = -4.0
_EXP_SHIFT_DVE = -2.0
_LOG2E = float(np.log2(np.e))
_A_E5 = 4.0 * _LOG2E * _PSC_SCALE              # DVE Schraudolph slope
_B_E5 = 60.0 + 4.0 * _LOG2E * _EXP_SHIFT_DVE

_PHASES = ("xT", "v", "all")


class EngineBalancer:
    """Greedy ACT/DVE assignment of PSUM drains by projected busy-ns."""

    def __init__(self, nc):
        self.nc = nc
        self.ns = {"act": 0.0, "dve": 0.0}

    def pick(self, act_cost, dve_cost):
        if self.ns["act"] + act_cost <= self.ns["dve"] + dve_cost:
            self.ns["act"] += act_cost
            return "act"
        self.ns["dve"] += dve_cost
        return "dve"

    def copy(self, out, in_, n_free):
        """PSUM fp32 -> SBUF copy with dtype convert, on the lighter engine."""
        eng = self.pick(n_free * 0.833 + 185, n_free * 1.042 + 125)
        if eng == "act":
            self.nc.scalar.copy(out, in_)
        else:
            self.nc.vector.tensor_copy(out, in_)

    def add_dve(self, cost):
        self.ns["dve"] += cost

    def add_act(self, cost):
        self.ns["act"] += cost


def build_bass(n_reps=1, phases="all", with_bias=True):
    _plevel = _PHASES.index(phases)
    nc = bacc.Bacc()

    x_d = nc.declare_dram_parameter("x", [S, D], bf16, isOutput=False)
    w_d = {
        "q": nc.declare_dram_parameter("Wq", [D, D], fp8e4, isOutput=False),
        "k": nc.declare_dram_parameter("Wk", [D, D], fp8e4, isOutput=False),
        "v": nc.declare_dram_parameter("Wv", [D, D], fp8e4, isOutput=False),
        "c": nc.declare_dram_parameter("Wc", [D, D], bf16, isOutput=False),
    }
    b_d = {}
    for nm in ("q", "k", "v", "c"):
        b_d[nm] = nc.declare_dram_parameter(f"b{nm}", [D], f32, isOutput=False)
    out_d = nc.declare_dram_parameter("out", [S, D], f32, isOutput=True)

    x3 = x_d.rearrange("(to p) d -> to p d", p=P)
    out3 = out_d.rearrange("(to p) d -> to p d", p=P)
    # fp8 weights viewed [p, kp, two, n]: DoubleRow pair (kp, two) on rows
    wv8 = {nm: w_d[nm].rearrange("(kp two p) n -> p kp two n", p=P, two=2)
           for nm in ("q", "k", "v")}
    wvc = w_d["c"].rearrange("(ko p) n -> p ko n", p=P)

    with tile.TileContext(nc) as tc:
        with (
            tc.tile_pool(name="const", bufs=1) as const_pool,
            tc.tile_pool(name="persist", bufs=1) as persist_pool,
            tc.tile_pool(name="scratch", bufs=3) as scratch_pool,
            tc.tile_pool(name="wqk", bufs=4) as wqk_pool,
            tc.tile_pool(name="wvc", bufs=4) as wvc_pool,
            tc.tile_pool(name="probs", bufs=3) as probs_pool,
            tc.tile_pool(name="outs", bufs=4) as outs_pool,
            tc.tile_pool(name="stage", bufs=2) as stage_pool,
            tc.tile_pool(name="small", bufs=4) as small_pool,
            tc.tile_pool(name="psum", bufs=4, space="PSUM") as psum_pool,
        ):
            for rep in range(n_reps):
                bal = EngineBalancer(nc)
                # ---------------- constants ----------------
                ident = const_pool.tile([P, P], f32)
                make_identity(nc, ident[:])
                identb = const_pool.tile([P, P], bf16)
                nc.vector.tensor_copy(identb[:], ident[:])
                exp_bias = const_pool.tile([P, 1], f32)
                nc.vector.memset(exp_bias[:], _EXP_SHIFT_ACT)

                # bias staging (feature-major for q/k; rows for v/c)
                bq_fm = const_pool.tile([P, KO], f32)
                bk_fm = const_pool.tile([P, KO], f32)
                ones_col = const_pool.tile([1, P], f32r)
                bv_row = const_pool.tile([1, D], f32r)
                bc_row = const_pool.tile([1, D], f32r)
                if with_bias:
                    nc.vector.memset(ones_col[:], 1.0)
                    for bname, bfm in (("q", bq_fm), ("k", bk_fm)):
                        brow8 = scratch_pool.tile([KO, P], f32, tag="brow8",
                                                  bufs=2, name=f"brow8_{bname}")
                        nc.sync.dma_start(
                            brow8[:],
                            b_d[bname].rearrange("(o p) -> o p", p=P))
                        pb = psum_pool.tile([P, KO], f32, tag="acc",
                                            name=f"pb_{bname}")
                        nc.tensor.transpose(pb[:], brow8[:], ident[:KO, :KO])
                        nc.vector.tensor_copy(bfm[:], pb[:])
                    nc.sync.dma_start(bv_row[:], b_d["v"][None, :])
                    nc.sync.dma_start(bc_row[:], b_d["c"][None, :])

                # ---------------- x -> xT (bf16) + xT8 (fp8) ----------------
                xT = persist_pool.tile([P, KO, S], bf16, tag="xT")
                xT8 = persist_pool.tile([P, KO, S], fp8e4, tag="xT8")
                xT8v = xT8.rearrange("p (kp two) s -> p kp two s", two=2)

                for to in range(TO):
                    x_raw = scratch_pool.tile([P, D], bf16, tag="raw2k",
                                              bufs=3, name=f"x_raw_{to}")
                    nc.sync.dma_start(x_raw[:, :512], x3[to][:, :512])
                    nc.sync.dma_start(x_raw[:, 512:], x3[to][:, 512:])
                    pt = psum_pool.tile([P, KO, P], bf16, tag="acc",
                                        name=f"pt_{to}")
                    for ko in range(KO):
                        nc.tensor.transpose(
                            pt[:, ko, :], x_raw[:, ko * P:(ko + 1) * P],
                            identb[:])
                    nc.vector.tensor_copy(
                        xT[:, :, to * P:(to + 1) * P], pt[:])
                    bal.add_dve(1024 * 0.52 + 125)
                # one big fp8 conversion (2x_2p)
                nc.vector.tensor_copy(
                    xT8.rearrange("p ko s -> p (ko s)"),
                    xT.rearrange("p ko s -> p (ko s)"))
                bal.add_dve(8192 * 0.52 + 60)

                if _plevel < 1:
                    continue
                # ------------- persistent projection outputs -------------
                qT8 = persist_pool.tile([P, KO, S], fp8e4, tag="qT8")
                kT8 = persist_pool.tile([P, KO, S], fp8e4, tag="kT8")
                vo = persist_pool.tile([P, TO, H * 65], fp8e4, tag="vo")
                vo5 = vo.rearrange("p to (h e) -> p to h e", e=65)
                nc.vector.memset(vo5[:, :, :, 64], WSCALE)
                attn_sb = persist_pool.tile([P, TO, H, W], bf16, tag="attn")

                def load_vc_w(nm, n):
                    """v (fp8 [p,kp,two,512]) or c (bf16 [p,ko,512]) half."""
                    if nm == "v":
                        w_r = wvc_pool.tile([P, KP, 2, 512], fp8e4,
                                            tag="wv_r", name=f"wv_{n}_{rep}")
                        nc.sync.dma_start(
                            w_r[:], wv8["v"][:, :, :, n * 512:(n + 1) * 512])
                    else:
                        w_r = wvc_pool.tile([P, KO, 512], bf16, tag="wc_r",
                                            name=f"wc_{n}_{rep}")
                        nc.sync.dma_start(
                            w_r[:], wvc[:, :, n * 512:(n + 1) * 512])
                    return w_r

                def qk_load(m):
                    w_rs = {}
                    for nm in ("q", "k"):
                        w_r = wqk_pool.tile([P, KP, 2, P], fp8e4, tag="wqk_r",
                                            name=f"wr_{nm}_{m}_{rep}")
                        nc.sync.dma_start(
                            w_r[:], wv8[nm][:, :, :, m * P:(m + 1) * P])
                        w_rs[nm] = w_r
                    return w_rs

                def v_unit(to, n, w_r):
                    """v token-major: out [128 tok, 512 feat] fp8 DR."""
                    ps = psum_pool.tile([P, 512], f32, tag="acc",
                                        name=f"ps_v_{n}_{to}_{rep}")
                    for kp in range(KP):
                        nc.tensor.matmul(
                            ps[:], xT8v[:, kp, :, to * P:(to + 1) * P],
                            w_r[:, kp, :, :], start=(kp == 0),
                            stop=(not with_bias and kp == KP - 1),
                            perf_mode=_DR)
                    if with_bias:
                        nc.tensor.matmul(
                            ps[:], ones_col[:],
                            bv_row[:, n * 512:(n + 1) * 512],
                            start=False, stop=True)
                    bal.copy(vo5[:, to, n * 8:(n + 1) * 8, :64],
                             ps.rearrange("p (h w) -> p h w", w=W), 512)

                def c_unit(to, n, w_r):
                    """c token-major bf16 + attn add + DMA out."""
                    ps = psum_pool.tile([P, 512], f32, tag="acc",
                                        name=f"ps_c_{n}_{to}_{rep}")
                    for ko in range(KO):
                        nc.tensor.matmul(
                            ps[:], xT[:, ko, to * P:(to + 1) * P],
                            w_r[:, ko, :], start=(ko == 0),
                            stop=(not with_bias and ko == KO - 1))
                    if with_bias:
                        nc.tensor.matmul(
                            ps[:], ones_col[:],
                            bc_row[:, n * 512:(n + 1) * 512],
                            start=False, stop=True)
                    yst = stage_pool.tile([P, 512], f32, tag="yst",
                                          name=f"yst_{n}_{to}_{rep}")
                    nc.vector.tensor_tensor(
                        yst.rearrange("p (h w) -> p h w", w=W),
                        ps.rearrange("p (h w) -> p h w", w=W),
                        attn_sb[:, to, n * 8:(n + 1) * 8, :],
                        _ALU_ADD)
                    bal.add_dve(512 * 1.042 + 190)
                    nc.sync.dma_start(
                        out3[to][:, n * 512:(n + 1) * 512], yst[:])

                def qk_emits(m, w_rs):
                    """Emit-callables for q/k projection chunk m: 2 names x
                    2 j-halves, each 4 DR matmuls + a fin drain."""
                    emits = []
                    for nm, dst, b_fm in (("q", qT8, bq_fm), ("k", kT8, bk_fm)):
                        for j in range(NJ):
                            state = {}

                            def _mk(nm=nm, dst=dst, b_fm=b_fm, j=j,
                                    state=state):
                                w_r = w_rs[nm]

                                def mm(kp, state=state):
                                    if kp == 0:
                                        state["ps"] = psum_pool.tile(
                                            [P, 512], f32, tag="acc",
                                            name=f"ps_{nm}_{m}_{j}_{rep}")
                                    nc.tensor.matmul(
                                        state["ps"][:], w_r[:, kp, :, :],
                                        xT8v[:, kp, :,
                                             j * 512:(j + 1) * 512],
                                        start=(kp == 0), stop=(kp == KP - 1),
                                        perf_mode=_DR)

                                def fin(state=state):
                                    dsl = dst[:, m, j * 512:(j + 1) * 512]
                                    if with_bias:
                                        nc.vector.tensor_scalar(
                                            dsl, state["ps"][:],
                                            b_fm[:, m:m + 1], None, _ALU_ADD)
                                        bal.add_dve(512 * 1.042 + 125)
                                    else:
                                        bal.copy(dsl, state["ps"][:], 512)

                                return ([lambda kp=kp: mm(kp)
                                         for kp in range(KP)] + [fin])

                            emits.extend(_mk())
                    return emits

                def qk_proj(m, w_rs=None):
                    if w_rs is None:
                        w_rs = qk_load(m)
                    for e in qk_emits(m, w_rs):
                        e()

                def attn_pair(hp, filler=None):
                    filler = list(filler or [])

                    def drain(k):
                        for _ in range(min(k, len(filler))):
                            filler.pop(0)()

                    heads = (2 * hp, 2 * hp + 1)
                    ots = {}
                    for j in range(NJ):
                        po = {h: psum_pool.tile([65, 512], f32, tag="acc",
                                                name=f"po_{h}_{j}_{rep}")
                              for h in heads}
                        # one engine per (h, j): the softmax rows of this
                        # unit live entirely on that engine's exp shift
                        eng_hj = {h: bal.pick(4 * (1024 * 0.833) + 4 * 185,
                                              4 * (1024 * 1.042) + 4 * 125)
                                  for h in heads}
                        for ib in range(TO // 2):
                            psc = {}
                            for h in heads:
                                p_lo = (h % 2) * W
                                psc[h] = psum_pool.tile(
                                    [P, 2, 512], f32, tag="acc2", bufs=2,
                                    name=f"psc_{h}_{j}_{ib}_{rep}")
                                for ii in range(2):
                                    i = ib * 2 + ii
                                    nc.tensor.matmul(
                                        psc[h][:, ii, :],
                                        kT8[p_lo:p_lo + W, hp,
                                            i * P:(i + 1) * P],
                                        qT8[p_lo:p_lo + W, hp,
                                            j * 512:(j + 1) * 512],
                                        start=True, stop=True)
                            drain(4)
                            for h in heads:
                                if eng_hj[h] == "act":
                                    probs = probs_pool.tile(
                                        [P, 2, 512], fp8e4, tag="probs",
                                        name=f"pr_{h}_{j}_{ib}_{rep}")
                                    nc.scalar.activation(
                                        probs[:], psc[h][:], _ACT_EXP,
                                        scale=_PSC_SCALE, bias=exp_bias[:])
                                    pr_mm = probs[:]
                                else:
                                    probs = probs_pool.tile(
                                        [P, 2, 512], u8, tag="probs",
                                        name=f"pr_{h}_{j}_{ib}_{rep}")
                                    nc.vector.tensor_scalar(
                                        probs[:], psc[h][:], _A_E5, _B_E5,
                                        _ALU_MULT, _ALU_ADD)
                                    pr_mm = probs[:].bitcast(fp8e5)
                                ib0 = ib * 2
                                nc.tensor.matmul(
                                    po[h][:], vo5[:, ib0:ib0 + 2, h, :],
                                    pr_mm, start=(ib == 0),
                                    stop=(ib == TO // 2 - 1), perf_mode=_DR)
                            drain(1)
                        for h in heads:
                            ot = outs_pool.tile([65, 512], bf16, tag="ot",
                                                name=f"ot_{h}_{j}_{rep}")
                            bal.copy(ot[:], po[h][:], 512)
                            ots[(h, j)] = ot
                    drain(len(filler))

                    # deferred normalize emits (run during the NEXT pair)
                    def _norm_one(h):
                        # 66-wide: keeps each [*, 65] PSUM write 4B-aligned
                        ptr = psum_pool.tile([P, NJ, 4, 66], bf16, tag="acc2",
                                             bufs=2, name=f"ptr_{h}_{rep}")
                        for j in range(NJ):
                            ot = ots[(h, j)]
                            for qo in range(4):
                                nc.tensor.transpose(
                                    ptr[:, j, qo, :65],
                                    ot[:, qo * P:(qo + 1) * P],
                                    identb[:65, :65])
                        recip = small_pool.tile([P, NJ, 4], f32, tag="recip",
                                                name=f"rc_{h}_{rep}")
                        nc.vector.reciprocal(recip[:], ptr[:, :, :, 64])
                        bal.add_dve(8 * 1.042 + 190)
                        nc.vector.tensor_tensor(
                            attn_sb.rearrange("p (j qo) h w -> p j qo h w",
                                              j=NJ)[:, :, :, h, :],
                            ptr[:, :, :, :64],
                            recip[:, :, :, None].to_broadcast((P, NJ, 4, W)),
                            _ALU_MULT)
                        bal.add_dve(512 * 1.042 + 190)

                    return [(lambda h=h: _norm_one(h)) for h in heads]

                # ------------- interleaved schedule -------------
                vw0 = load_vc_w("v", 0)
                for to in range(TO):
                    v_unit(to, 0, vw0)
                if _plevel < 2:
                    continue
                cw = {}
                vw1 = None
                qk_proj(0)
                pending_norm = []
                for hp in range(H // 2):
                    if hp == 1:
                        vw1 = load_vc_w("v", 1)
                    if hp + 1 < H // 2:
                        nxt = qk_emits(hp + 1, qk_load(hp + 1))
                    else:
                        nxt = []
                    pending_norm = attn_pair(hp, filler=pending_norm + nxt)
                    if hp == 3:
                        for to in range(TO):
                            v_unit(to, 1, vw1)
                        cw[0] = load_vc_w("c", 0)
                    if hp >= 4:
                        for to2 in range(2):
                            to = (hp - 4) * 2 + to2
                            c_unit(to, 0, cw[0])
                for e in pending_norm:
                    e()
                cw[1] = load_vc_w("c", 1)
                for to in range(TO):
                    c_unit(to, 1, cw[1])

    nc.compile()
    return nc


_NC_CACHE = {}


def _get_nc(with_bias=True):
    if with_bias not in _NC_CACHE:
        _NC_CACHE[with_bias] = build_bass(with_bias=with_bias)
    return _NC_CACHE[with_bias]


def _reference_numpy(x, splits, Wq, bq, Wk, bk, Wv, bv, Wc, bc):
    """Exact fallback for unexpected (non-equal) segmentations."""
    x = x.astype(np.float64)
    q = x @ Wq + bq
    c = x @ Wc + bc
    k = x @ Wk + bk
    v = x @ Wv + bv
    T, Dm = x.shape
    Wh = Dm // H
    out = np.empty_like(x)
    for s0, s1 in np.asarray(splits):
        qs = q[s0:s1].reshape(s1 - s0, H, Wh)
        ks = k[s0:s1].reshape(s1 - s0, H, Wh)
        vs = v[s0:s1].reshape(s1 - s0, H, Wh)
        sc = np.einsum("qhw,khw->hqk", qs, ks) / np.sqrt(Wh)
        sc -= sc.max(axis=-1, keepdims=True)
        e = np.exp(sc)
        pr = e / e.sum(axis=-1, keepdims=True)
        out[s0:s1] = np.einsum("hqk,khw->qhw", pr, vs).reshape(s1 - s0, Dm)
    return (out + c).astype(np.float32)


def _pack_args(Wq, bq, Wk, bk, Wv, bv, Wc, bc):
    out = {}
    for k, v in (("Wq", Wq), ("Wk", Wk), ("Wv", Wv)):
        out[k] = np.ascontiguousarray(
            np.asarray(v, np.float32) * WSCALE).astype(ml_dtypes.float8_e4m3)
    out["Wc"] = np.ascontiguousarray(
        np.asarray(Wc, np.float32)).astype(ml_dtypes.bfloat16)
    # q/k biases are added post-matmul in the 32x-scaled domain
    out["bq"] = np.ascontiguousarray(np.asarray(bq, np.float32) * WSCALE)
    out["bk"] = np.ascontiguousarray(np.asarray(bk, np.float32) * WSCALE)
    out["bv"] = np.ascontiguousarray(np.asarray(bv, np.float32) * WSCALE)
    out["bc"] = np.ascontiguousarray(np.asarray(bc, np.float32))
    return out


def _in_maps(x, args):
    return [
        {"x": x[i * S:(i + 1) * S],
         **{f"W{nm}": args[f"W{nm}"] for nm in "qkvc"},
         **{f"b{nm}": args[f"b{nm}"] for nm in "qkvc"}}
        for i in range(NCORES)
    ]


def kernel(x, splits, Wq, bq, Wk, bk, Wv, bv, Wc, bc):
    x = np.ascontiguousarray(x, dtype=np.float32)

    sp = np.asarray(splits)
    expected = np.stack(
        [np.arange(NCORES) * S, (np.arange(NCORES) + 1) * S], axis=1
    )
    if sp.shape != (NCORES, 2) or not np.array_equal(
        sp.astype(np.int64), expected.astype(np.int64)
    ):
        return _reference_numpy(
            x, sp,
            np.asarray(Wq, np.float64), np.asarray(bq, np.float64),
            np.asarray(Wk, np.float64), np.asarray(bk, np.float64),
            np.asarray(Wv, np.float64), np.asarray(bv, np.float64),
            np.asarray(Wc, np.float64), np.asarray(bc, np.float64))

    args = _pack_args(Wq, bq, Wk, bk, Wv, bv, Wc, bc)
    xb = x.astype(ml_dtypes.bfloat16)

    need_bias = any(np.any(args[f"b{nm}"]) for nm in "qkvc")
    r = run_bass_kernel_spmd(_get_nc(need_bias), _in_maps(xb, args),
                             list(range(NCORES)))
    return np.concatenate([r.results[i]["out"] for i in range(NCORES)],
                          axis=0)
